# revision 32
# baseline (speedup 1.0000x reference)
"""Trainium2 Bass kernel for nn_DiscreteAutoregressiveFlow (sampling, forward).

Math: `inputs` is an exact one-hot [B, L, V] tensor. For a row holding token v:
  net = W[v] + b                      (exact: one-hot @ W picks a row)
  loc = one_hot(argmax(net[:V]));  scale = one_hot(argmax(net[V:]))
  one_hot_multiply -> one-hot at (scale_tok*v) % V   (zero row if scale_tok==0)
  one_hot_add      -> one-hot at (scale_tok*v + loc_tok) % V
So out[row] = one_hot(cmap[v]), i.e. out = x @ T for the 0/1 matrix
T[v, cmap[v]] = 1 (zero row for scale_tok==0). The straight-through softmax
residuals and FFT noise in the reference are O(1e-7) and vanish in norm rel
error.

Device pipeline (per core, pure streaming, memory-bound):
  The host uploads x TRANSPOSED and 2-stacked, fp8e4 (0.0/1.0 exact in fp8),
  plus the stationary weight blockdiag(T, T) [128, 128] fp8. Per N=512 slice:
    psum f32 = Tbd.T @ xs[:, n:n+512]      (TensorE matmul, exact 0/1 sums)
    sbuf fp8 <- psum                       (ACT/DVE copy-cast, 2 banks/copy)
    DMA out fp8 chunks
  All input DMAs are issued up-front on the Sync queue (tiny tbd const
  first to absorb the first-DMA cold cost; output-DMA waits cannot delay
  input issue); first/last chunks are small to shorten the critical-path
  head and tail; 8 dummy matmuls on a memset scratch tile run during the
  first DMA's ~4us latency so the HAM clock-gate is warm (2.4 GHz) for the
  real matmul stream; the final group's copy and output DMA issue from the
  same (Scalar) queue to skip a cross-engine semaphore hop.
  Host un-transposes and casts fp8 -> f32 (exact for 0/1 values).
HBM traffic per core: 1MB in + 1MB out (vs 8.4MB for f32 I/O).
Sharding: pure data parallel over B*L rows, 8 cores, no collectives.
"""

import numpy as np
import ml_dtypes

V = 64
P = 128
N_CORES = 8
B, L = 16, 8192
ROWS = B * L                      # 131072
ROWS_PER_CORE = ROWS // N_CORES   # 16384
HALF = ROWS_PER_CORE // 2         # 8192 columns in transposed layout

# Column counts per input chunk. Each chunk's matmul outputs go through
# 2-bank (or 1-bank) PSUM tiles; first/last chunks are small to shorten the
# critical path head (first matmul) and tail (last copy + output DMA).
CHUNKS = (512, 1024, 2048, 2048, 1536, 512, 512)
N_DUMMY = 8                       # PE warm-up matmuls

_F8 = ml_dtypes.float8_e4m3

_CACHE = {}


def _build_nc(chunks=CHUNKS, n_dummy=N_DUMMY):
    import concourse.bacc as bacc
    import concourse.mybir as mybir
    from concourse.tile import TileContext

    f8 = mybir.dt.float8e4
    f32 = mybir.dt.float32
    assert sum(chunks) == HALF

    # Bacc (not raw Bass): its compile() runs generate_event_semaphores(),
    # which legalizes multi-wait instructions for TRN2 (1 wait per instr).
    nc = bacc.Bacc("TRN2", target_bir_lowering=False, name="daf_mm")
    xs = nc.dram_tensor("xs", [P, HALF], f8, kind="ExternalInput")
    tbd = nc.dram_tensor("tbd", [P, P], f8, kind="ExternalInput")
    y = nc.dram_tensor("y", [P, HALF], f8, kind="ExternalOutput")

    with TileContext(nc) as tc:
        with (
            tc.tile_pool(name="const", bufs=1) as constp,
            tc.tile_pool(name="xin", bufs=len(chunks)) as xinp,
            tc.tile_pool(name="yout", bufs=len(chunks)) as youtp,
            tc.tile_pool(name="warm", bufs=1) as warmp,
            tc.tile_pool(name="ps", bufs=4, space="PSUM") as psp,
        ):
            # Warm-up scratch memset first so the dummy matmuls can begin as
            # early as possible (gpsimd is otherwise idle).
            scratch = warmp.tile([P, 640], f8, tag="scratch")
            nc.gpsimd.memset(scratch[:], 0)

            # All input DMAs up-front, back-to-back on the Sync queue (the
            # Scalar queue's head would be blocked by the auto-inserted
            # ACT_TABLE_LOAD). Tiny tbd const first absorbs the first-DMA
            # cold cost and is ready before the first real LDWEIGHTS.
            t_sb = constp.tile([P, P], f8, tag="t_sb")
            nc.sync.dma_start(t_sb[:], tbd[:])
            x_sb = [xinp.tile([P, cols], f8, name=f"x{ci}", tag=f"x{ci}")
                    for ci, cols in enumerate(chunks)]
            offs = [sum(chunks[:ci]) for ci in range(len(chunks))]
            for ci in range(len(chunks)):
                nc.sync.dma_start(x_sb[ci][:], xs[:, offs[ci]:offs[ci] + chunks[ci]])

            # Warm the PE clock gate during the first chunk's DMA latency.
            # The dummy PSUM tile shares the pool so all 4 bufs = 8 banks
            # serve the real pipeline (real MM overwrites with start=True).
            ps_d = psp.tile([P, 1024], f32, tag="ps")
            for _ in range(n_dummy):
                nc.tensor.matmul(ps_d[:, :512], scratch[:, :P],
                                 scratch[:, P:640], start=True, stop=True)

            # PSUM->SBUF cast engine per group: alternate ACT/DVE (gpsimd
            # has no PSUM port); the final group lands on ACT so the final
            # output DMA can issue from the same queue with no cross-engine
            # hop, and the penultimate on DVE so they overlap.
            n_groups = sum((cols + 1023) // 1024 for cols in chunks)
            cp_eng = {}
            for g in range(n_groups):
                cp_eng[g] = nc.scalar if g % 2 == 0 else nc.vector
            cp_eng[n_groups - 2] = nc.vector
            cp_eng[n_groups - 1] = nc.scalar

            k = 0
            for ci, cols in enumerate(chunks):
                y_sb = youtp.tile([P, cols], f8, tag=f"y{ci}")
                for h in range((cols + 1023) // 1024):
                    gcols = min(1024, cols - h * 1024)
                    ps = psp.tile([P, 1024], f32, tag="ps")
                    for q in range((gcols + 511) // 512):
                        lo = h * 1024 + q * 512
                        n = min(512, gcols - q * 512)
                        nc.tensor.matmul(
                            ps[:, q * 512:q * 512 + n], t_sb[:],
                            x_sb[ci][:, lo:lo + n], start=True, stop=True,
                        )
                    dst = y_sb[:, h * 1024:h * 1024 + gcols]
                    eng = cp_eng[k]
                    if eng is nc.scalar:
                        eng.copy(dst, ps[:, :gcols])
                    else:
                        eng.tensor_copy(dst, ps[:, :gcols])
                    k += 1

                # Final chunk's output DMA on the Scalar queue right after
                # its copy; the rest on Sync.
                eng = nc.scalar if ci == len(chunks) - 1 else nc.sync
                eng.dma_start(y[:, offs[ci]:offs[ci] + cols], y_sb[:])

    # Bacc.finalize runs compile(): wait-splitting (generate_event_semaphores),
    # register allocation, nop fusion. run_bass_via_pjrt serializes nc.m as-is,
    # so this must happen here.
    nc.finalize()
    return nc


def _get_nc(chunks=CHUNKS, n_dummy=N_DUMMY):
    key = (chunks, n_dummy)
    if key not in _CACHE:
        _CACHE[key] = _build_nc(chunks, n_dummy)
    return _CACHE[key]


def _host_tmat(W: np.ndarray, b: np.ndarray) -> np.ndarray:
    """[128, 128] fp8 blockdiag(T, T); T[v, cmap[v]] = 1, zero row if
    scale_tok == 0."""
    net = W.astype(np.float32) + b.astype(np.float32)[None, :]   # [V, 2V]
    loc_tok = np.argmax(net[:, :V], axis=1)                      # [V]
    scale_tok = np.argmax(net[:, V:], axis=1)                    # [V]
    t = (scale_tok * np.arange(V, dtype=np.int64) + loc_tok) % V
    T = np.zeros((V, V), dtype=np.float32)
    nz = scale_tok != 0
    T[np.arange(V)[nz], t[nz]] = 1.0
    tbd = np.zeros((P, P), dtype=np.float32)
    tbd[:V, :V] = T
    tbd[V:, V:] = T
    return tbd.astype(_F8)


def _prep_in_maps(inputs: np.ndarray, W: np.ndarray, b: np.ndarray):
    """Shard + transpose-stack + fp8-cast the full one-hot input."""
    x8 = np.asarray(inputs, dtype=np.float32).reshape(ROWS, V).astype(_F8)
    # [cores, 2, HALF, V] -> [cores, 2, V, HALF] -> [cores, 128, HALF]
    xt = np.ascontiguousarray(
        x8.reshape(N_CORES, 2, HALF, V).transpose(0, 1, 3, 2)
    ).reshape(N_CORES, P, HALF)
    tbd = _host_tmat(W, b)
    return [{"xs": xt[c], "tbd": tbd} for c in range(N_CORES)]


def _post(results, dtype, shape):
    yd = np.stack([np.asarray(r["y"]) for r in results])          # [8, 128, HALF]
    y = yd.reshape(N_CORES, 2, V, HALF).transpose(0, 1, 3, 2)     # -> rows, V
    return np.ascontiguousarray(y).reshape(shape).astype(dtype, copy=False)


def kernel(inputs: np.ndarray, W: np.ndarray, b: np.ndarray) -> np.ndarray:
    from concourse import bass_utils

    in_maps = _prep_in_maps(inputs, W, b)
    nc = _get_nc()
    res = bass_utils.run_bass_kernel_spmd(nc, in_maps, core_ids=list(range(N_CORES)))
    return _post(res.results, inputs.dtype, inputs.shape)
